# revision 8
# baseline (speedup 1.0000x reference)
"""Colight GNN message-passing kernel for 8x TRN2 NeuronCores (Bass/Tile).

Math (per head h of 5, agent b, hidden d of 128, neighbors n of deg(b)<=8):
    actor = x @ Wa_h + ba_h                     [B, 128]
    oth_n = others_n @ Wo_h + bo_h              (valid neighbors only)
    y     = actor * oth_n ; e = exp(y)
    s     = sum_n e ; num = sum_n e * (others_n @ Wf_h + bf_h)
    out   = relu((mean_h num/s) @ W_fc + b_fc)

Key structure (all chosen against the TRN2 engine cost model):
  * Degree compaction: host sorts agents by neighbor count (striped across
    cores so every core sees the same degree histogram), packs each agent's
    valid neighbors densely, and pads chunks to a common per-chunk K.  All
    n-proportional work (matmul, exp, vector ops, DMA) scales by mean
    deg/8 ~= 0.56.
  * Mask elimination: masked/padded slots have host-zeroed features and a
    0 in the augmented bias row, so oth=0 -> e=exp(0)=1 exactly and fin=0.
    The denominator is fixed by subtracting the per-agent dummy count c
    (only in chunks that mix degrees).
  * Biases ride the matmul: inputs carry a 65th row (mask/ones) and the
    weights a 65th row (bias), so no bias pass on vector engines.
  * Softmax sums on the PE: s = sum_n e and num = sum_n p are computed by
    K accumulating identity matmuls into PSUM instead of DVE reduce trees.
  * fp16 intermediates everywhere; PSUM accumulation stays fp32.

Per-core layout: d=128 on partitions, agents on the free axis, chunks of
BC=512 agents, degree-ascending.
"""

import math
import sys

import numpy as np

sys.path.insert(0, "/opt/trn_rl_repo")

import concourse.bass as bass
import concourse.tile as tile
from concourse import mybir
from concourse.vector_clock import ScopedClock


# ---------------------------------------------------------------- walrus quirk
def _patched_drain_and_barrier(self, tick_clock, wait_clock):
    """Kernel-tail drain: this walrus build rejects an instruction carrying
    many sem waits ("Too many sync wait commands"), so spread the final
    clock-sync waits across single-wait nops before the drain."""
    nop_inst = self.nc.sync.nop(nofuse=True)
    wait_clock.add_sem_waits(
        nop_inst.ins, ScopedClock({None: tick_clock.global_clock})
    )
    waits = list(nop_inst.ins.sync_info.on_wait or [])
    if len(waits) > 1:
        nop_inst.ins.sync_info.on_wait = waits[:1]
        for w in waits[1:]:
            extra = self.nc.sync.nop(nofuse=True)
            if extra.ins.sync_info is None:
                extra.ins.sync_info = mybir.SyncInfo(on_wait=[], on_update=[])
            extra.ins.sync_info.on_wait = [w]
    self.nc.sync.drain()
    self.nc.all_engine_barrier()
    assert self.sems is not None
    popped = self.nc._tile_sem_poison_stack.pop()
    assert popped is self._sem_poison
    self.nc.clear_and_free_semaphores(list(self.sems.allocated().values()))
    self.nc.all_engine_barrier()


tile.TileContext._drain_and_barrier = _patched_drain_and_barrier

_WAIT_LIMIT = 1  # this walrus build rejects >1 sem wait per instruction


def _split_sync_waits(nc):
    """Move excess sem waits from any instruction onto preceding nofuse
    nops on the same engine (walrus 'Too many sync wait commands')."""
    n_split = 0
    for fn in nc.m.functions:
        for blk in fn.blocks:
            new_insts = []
            for inst in blk.instructions:
                si = inst.sync_info
                if si is not None and si.on_wait and len(si.on_wait) > _WAIT_LIMIT:
                    waits = list(si.on_wait)
                    for w in waits[:-_WAIT_LIMIT]:
                        n_split += 1
                        new_insts.append(
                            mybir.InstNoOp(
                                name=f"waitsplit_{n_split}_{inst.name}",
                                engine=inst.engine,
                                sync_info=mybir.SyncInfo(
                                    on_wait=[w], on_update=[]
                                ),
                                bass_nofuse=True,
                                text_hint="waitsplit",
                            )
                        )
                    si.on_wait = waits[-_WAIT_LIMIT:]
                new_insts.append(inst)
            blk.instructions = new_insts


# ------------------------------------------------------------------- constants
B, N_NEI, IN_DIM, HEADS, HID = 100000, 8, 64, 5, 128
NCORES = 8
BSH = B // NCORES          # 12500 agents per core
BC = 512                   # agents per chunk
NCH = math.ceil(BSH / BC)  # 25
PB = NCH * BC              # 12800 padded agents per core
AUG = IN_DIM + 1           # 65: 64 features + mask/bias row

AF = mybir.ActivationFunctionType
ALU = mybir.AluOpType
F32 = mybir.dt.float32
F16 = mybir.dt.float16


# ------------------------------------------------------------------- host plan
def compute_plan(neighbor_masks):
    """Degree-sort agents, stripe across cores, derive the common per-chunk
    neighbor-count profile.

    Returns dict with:
      cores:    [NCORES][BSH] global agent ids, degree-ascending per core
      deg:      [B] neighbor counts
      K:        [NCH] per-chunk neighbor slots (same for all cores)
      cidx:     [NCH] -> row into the c matrix, or -1 if no correction needed
      nb:       number of correction rows (>=1 for the dram tensor)
    """
    m = np.asarray(neighbor_masks) != 0
    deg = m.sum(axis=0).astype(np.int64)  # [B] in 1..8
    order = np.argsort(deg, kind="stable")
    cores = [order[c::NCORES] for c in range(NCORES)]  # each ascending in deg

    K = np.zeros(NCH, np.int64)
    needs_c = np.zeros(NCH, bool)
    for ch in range(NCH):
        sl = slice(ch * BC, (ch + 1) * BC)
        kmax = 1
        for c in range(NCORES):
            d = deg[cores[c][sl]]
            if d.size:
                kmax = max(kmax, int(d.max()))
        K[ch] = kmax
        for c in range(NCORES):
            d = deg[cores[c][sl]]
            if d.size < BC or (d.size and int(d.min()) < kmax):
                needs_c[ch] = True
    cidx = np.full(NCH, -1, np.int64)
    nxt = 0
    for ch in range(NCH):
        if needs_c[ch]:
            cidx[ch] = nxt
            nxt += 1
    return {"cores": cores, "deg": deg, "K": K, "cidx": cidx, "nb": max(nxt, 1)}


def prep_host_inputs(plan, x, others, neighbor_masks, Wa, ba, Wo, bo, Wf, bf,
                     W_fc, b_fc):
    x = np.asarray(x, np.float32)
    others = np.asarray(others, np.float32)
    m = np.asarray(neighbor_masks) != 0
    deg, K, cidx = plan["deg"], plan["K"], plan["cidx"]
    totk = int(K.sum())

    # compact valid neighbors to slots 0..deg-1 (host side, fp16)
    pos = np.cumsum(m, axis=0) - 1  # [8, B] slot index where valid
    comp = np.zeros((N_NEI, B, IN_DIM), np.float16)
    mrow = np.zeros((N_NEI, B), np.float16)
    nn, bb = np.nonzero(m)
    comp[pos[nn, bb], bb] = others[nn, bb]
    mrow[pos[nn, bb], bb] = 1.0

    w3 = np.empty((3, HEADS, AUG, HID), np.float16)
    for t, (W, bias) in enumerate(((Wa, ba), (Wo, bo), (Wf, bf))):
        w3[t, :, :IN_DIM, :] = np.asarray(W, np.float32)
        w3[t, :, IN_DIM, :] = np.asarray(bias, np.float32)
    ident = np.eye(HID, dtype=np.float16)
    wfc = (np.asarray(W_fc, np.float32) / HEADS).astype(np.float16)
    bfc = np.asarray(b_fc, np.float32).reshape(HID, 1)

    in_maps = []
    for c in range(NCORES):
        ag = plan["cores"][c]
        degc = np.ones(PB, np.int64)
        degc[:BSH] = deg[ag]
        xc = np.zeros((PB, IN_DIM), np.float16)
        xc[:BSH] = x[ag]
        compc = np.zeros((N_NEI, PB, IN_DIM), np.float16)
        compc[:, :BSH] = comp[:, ag]
        mrowc = np.zeros((N_NEI, PB), np.float16)
        mrowc[:, :BSH] = mrow[:, ag]
        mrowc[0, BSH:] = 1.0  # pad agents: one dummy neighbor (deg 1)

        xT = np.empty((AUG, PB), np.float16)
        xT[:IN_DIM] = xc.T
        xT[IN_DIM] = 1.0

        slab = np.zeros((totk, AUG, BC), np.float16)
        cmat = np.zeros((plan["nb"], BC), np.float32)
        ptr = 0
        for ch in range(NCH):
            kc = int(K[ch])
            sl = slice(ch * BC, (ch + 1) * BC)
            slab[ptr:ptr + kc, :IN_DIM, :] = compc[:kc, sl, :].transpose(0, 2, 1)
            slab[ptr:ptr + kc, IN_DIM, :] = mrowc[:kc, sl]
            if cidx[ch] >= 0:
                cmat[cidx[ch]] = (kc - degc[sl]).astype(np.float32)
            ptr += kc

        in_maps.append({
            "xT": xT,
            "oth": slab,
            "cmat": cmat,
            "w3": w3,
            "ident": ident,
            "wfc": wfc,
            "bfc": bfc,
        })
    return in_maps


# ---------------------------------------------------------------- device build
# Work-assignment knobs: per-head engine choices, tuned against TimelineSim.
# P_POOL[h]: p-mult on gpsimd instead of DVE.  FIN_DVE[h]: fin PSUM->SBUF
# evac on DVE (tensor_copy) instead of ACT.
P_POOL = (False, False, False, False, False)
FIN_DVE = (False, False, False, False, False)


def build_nc(plan, repeat: int = 1, split_waits: bool = True):
    K, cidx = plan["K"], plan["cidx"]
    totk = int(K.sum())
    nc = bass.Bass("TRN2", target_bir_lowering=False, debug=False)

    xT = nc.dram_tensor("xT", [AUG, PB], F16, kind="ExternalInput").ap()
    oth = nc.dram_tensor("oth", [totk, AUG, BC], F16, kind="ExternalInput").ap()
    cmat = nc.dram_tensor("cmat", [plan["nb"], BC], F32, kind="ExternalInput").ap()
    w3 = nc.dram_tensor("w3", [3, HEADS, AUG, HID], F16, kind="ExternalInput").ap()
    ident = nc.dram_tensor("ident", [HID, HID], F16, kind="ExternalInput").ap()
    wfc = nc.dram_tensor("wfc", [HID, HID], F16, kind="ExternalInput").ap()
    bfc = nc.dram_tensor("bfc", [HID, 1], F32, kind="ExternalInput").ap()
    out = nc.dram_tensor("out", [HID, PB], F32, kind="ExternalOutput").ap()

    with tile.TileContext(nc) as tc:
        with (
            tc.tile_pool(name="singles", bufs=1) as singles,
            tc.tile_pool(name="io", bufs=2) as io,
            tc.tile_pool(name="work", bufs=3) as work,
            tc.tile_pool(name="small", bufs=3) as small,
            tc.tile_pool(name="ps_oth", bufs=1, space="PSUM") as ps_oth,
            tc.tile_pool(name="ps_fin", bufs=1, space="PSUM") as ps_fin,
            tc.tile_pool(name="ps_sfc", bufs=2, space="PSUM") as ps_sfc,
            tc.tile_pool(name="ps_a", bufs=1, space="PSUM") as ps_a,
            tc.tile_pool(name="ps_num", bufs=1, space="PSUM") as ps_num,
        ):
            w_sb = singles.tile([AUG, 3, HEADS, HID], F16)
            nc.sync.dma_start(out=w_sb, in_=w3.rearrange("t h k d -> k t h d"))
            id_sb = singles.tile([HID, HID], F16)
            nc.sync.dma_start(out=id_sb, in_=ident)
            wfc_sb = singles.tile([HID, HID], F16)
            nc.sync.dma_start(out=wfc_sb, in_=wfc)
            bfc_sb = singles.tile([HID, 1], F32)
            nc.sync.dma_start(out=bfc_sb, in_=bfc)

            for _rep in range(repeat):
                ptr = 0
                for ch in range(NCH):
                    kc = int(K[ch])
                    b0 = ch * BC
                    x_sb = io.tile([AUG, BC], F16, tag="x_sb")
                    nc.sync.dma_start(out=x_sb, in_=xT[:, b0:b0 + BC])
                    oth_sb = io.tile([AUG, N_NEI, BC], F16, tag="oth_sb")
                    nc.sync.dma_start(
                        out=oth_sb[:, :kc, :],
                        in_=oth[ptr:ptr + kc].rearrange("k a b -> a k b"),
                    )
                    if cidx[ch] >= 0:
                        c_sb = small.tile([HID, BC], F32, tag="c_sb")
                        csl = cmat[cidx[ch]:cidx[ch] + 1, :]
                        nc.sync.dma_start(
                            out=c_sb,
                            in_=bass.AP(
                                tensor=csl.tensor,
                                offset=csl.offset,
                                ap=[[0, HID]] + list(csl.ap)[1:],
                            ),
                        )
                    ptr += kc

                    nr_sb = small.tile([HID, HEADS, BC], F16, tag="nr_sb")

                    for h in range(HEADS):
                        # actor
                        na_a = ps_a.tile([HID, BC], F32, tag="a")
                        nc.tensor.matmul(
                            out=na_a, lhsT=w_sb[:, 0, h, :], rhs=x_sb,
                            start=True, stop=True,
                        )
                        a_sb = small.tile([HID, BC], F16, tag="a_sb")
                        nc.scalar.copy(out=a_sb, in_=na_a)

                        # oth projections in groups of 2, y per group
                        y_sb = work.tile([HID, N_NEI, BC], F16, tag="y")
                        for g0 in range(0, kc, 2):
                            ng = min(2, kc - g0)
                            po = ps_oth.tile([HID, 2, BC], F32, tag="po")
                            for j in range(ng):
                                nc.tensor.matmul(
                                    out=po[:, j, :],
                                    lhsT=w_sb[:, 1, h, :],
                                    rhs=oth_sb[:, g0 + j, :],
                                    start=True, stop=True,
                                )
                            a_bc = a_sb.rearrange(
                                "p (o b) -> p o b", o=1
                            ).to_broadcast([HID, ng, BC])
                            nc.vector.tensor_mul(
                                y_sb[:, g0:g0 + ng, :], po[:, :ng, :], a_bc
                            )

                        # e = exp(y), one batched ACT op
                        e_sb = work.tile([HID, N_NEI, BC], F16, tag="e")
                        nc.scalar.activation(
                            out=e_sb[:, :kc, :], in_=y_sb[:, :kc, :], func=AF.Exp
                        )

                        # s = sum_n e via accumulating identity matmuls
                        s_ps = ps_sfc.tile([HID, BC], F32, tag="sfc")
                        for n in range(kc):
                            nc.tensor.matmul(
                                out=s_ps, lhsT=id_sb, rhs=e_sb[:, n, :],
                                start=(n == 0), stop=(n == kc - 1),
                            )

                        # fin projections in groups of 2, ACT evac to fp16
                        fin_sb = work.tile([HID, N_NEI, BC], F16, tag="fin")
                        for g0 in range(0, kc, 2):
                            ng = min(2, kc - g0)
                            pf = ps_fin.tile([HID, 2, BC], F32, tag="pf")
                            for j in range(ng):
                                nc.tensor.matmul(
                                    out=pf[:, j, :],
                                    lhsT=w_sb[:, 2, h, :],
                                    rhs=oth_sb[:, g0 + j, :],
                                    start=True, stop=True,
                                )
                            if FIN_DVE[h]:
                                nc.vector.tensor_copy(
                                    out=fin_sb[:, g0:g0 + ng, :], in_=pf[:, :ng, :]
                                )
                            else:
                                nc.scalar.copy(
                                    out=fin_sb[:, g0:g0 + ng, :], in_=pf[:, :ng, :]
                                )

                        # p = e * fin, one batched op (DVE or gpsimd)
                        p_sb = work.tile([HID, N_NEI, BC], F16, tag="p")
                        p_eng = nc.gpsimd if P_POOL[h] else nc.vector
                        p_eng.tensor_mul(
                            p_sb[:, :kc, :], e_sb[:, :kc, :], fin_sb[:, :kc, :]
                        )

                        # num = sum_n p via accumulating identity matmuls
                        num_ps = ps_num.tile([HID, BC], F32, tag="num")
                        for n in range(kc):
                            nc.tensor.matmul(
                                out=num_ps, lhsT=id_sb, rhs=p_sb[:, n, :],
                                start=(n == 0), stop=(n == kc - 1),
                            )

                        # r = 1/(s - c); nr = num * r
                        r_sb = small.tile([HID, BC], F32, tag="r_sb")
                        if cidx[ch] >= 0:
                            sp_sb = small.tile([HID, BC], F32, tag="sp_sb")
                            nc.vector.tensor_sub(sp_sb, s_ps, c_sb)
                            nc.vector.reciprocal(out=r_sb, in_=sp_sb)
                        else:
                            nc.vector.reciprocal(out=r_sb, in_=s_ps)
                        nc.vector.tensor_mul(nr_sb[:, h, :], num_ps, r_sb)

                    # fc + relu + store
                    fc_ps = ps_sfc.tile([HID, BC], F32, tag="sfc")
                    for h in range(HEADS):
                        nc.tensor.matmul(
                            out=fc_ps, lhsT=wfc_sb, rhs=nr_sb[:, h, :],
                            start=(h == 0), stop=(h == HEADS - 1),
                        )
                    out_sb = small.tile([HID, BC], F32, tag="out_sb")
                    nc.scalar.activation(
                        out=out_sb, in_=fc_ps, func=AF.Relu, bias=bfc_sb
                    )
                    nc.sync.dma_start(out=out[:, b0:b0 + BC], in_=out_sb)

    if split_waits:
        _split_sync_waits(nc)
    return nc


# --------------------------------------------------------------------- runtime
_CACHE = {}


def _plan_key(plan):
    return (tuple(int(k) for k in plan["K"]), tuple(int(i) for i in plan["cidx"]))


def _get_nc(plan, repeat=1):
    key = (_plan_key(plan), repeat)
    if key not in _CACHE:
        _CACHE[key] = build_nc(plan, repeat=repeat)
    return _CACHE[key]


def kernel(**inputs):
    from concourse.bass_utils import run_bass_kernel_spmd

    inputs = {k: np.asarray(v) for k, v in inputs.items()}
    plan = compute_plan(inputs["neighbor_masks"])
    in_maps = prep_host_inputs(plan, **inputs)
    res = run_bass_kernel_spmd(
        _get_nc(plan), in_maps, core_ids=list(range(NCORES))
    )
    full = np.empty((B, HID), np.float32)
    for c in range(NCORES):
        o = res.results[c]["out"]  # [HID, PB]
        full[plan["cores"][c]] = o[:, :BSH].T
    return full


def profile_exec_ns(inputs):
    from concourse.bass_utils import run_bass_kernel_spmd

    plan = compute_plan(inputs["neighbor_masks"])
    in_maps = prep_host_inputs(plan, **inputs)
    res = run_bass_kernel_spmd(
        _get_nc(plan), in_maps, core_ids=list(range(NCORES)), trace=True
    )
    return res.exec_time_ns
